# revision 80
# baseline (speedup 1.0000x reference)
"""HSTU block kernel for Trainium2, 8-core data-parallel over batch.

Layouts are chosen so no on-device transposes are needed:
  - x is shipped both as xT [D, N] (for stats + matmul rhs) and as
    xr_pre = (x + b_o) * (1-pad) row-major (residual + bias fold).
  - all big matmuls run in f16 (weights pre-cast host-side, activations cast
    on device); PSUM accumulation stays fp32.
  - proj is produced transposed (projT [E, N]) for u/q/k; v is produced
    row-major [N, DV*H] so it can be the stationary operand of the attn@v
    matmul.
  - qk logits are produced transposed (LT [key m, query n]); the rel-bias is
    accumulated in the same [m, n] layout (f16 master acc seeded from a host
    pos-bias table) and preloaded into PSUM via an f16 identity matmul so the
    qk matmul accumulates on top of it.  The causal mask inside the diagonal
    128-blocks is applied by writing -1e4 into the masked cells of the bias
    once per row-tile: silu(-1e4 + qk) underflows to 0.
  - ts_w[bucket(log(dt))] is piecewise-constant in log(dt): reconstructed with
    threshold-compare/accumulate passes on DVE (thresholds/coefs baked as
    immediates at build time; per-chunk threshold ranges pruned using the
    actual timestamp ranges, unioned across the 8 batches so one SPMD program
    works for all cores).
"""

import sys

sys.path.insert(0, "/opt/trn_rl_repo")

import numpy as np

import concourse.bass as bass
import concourse.tile as tile
import concourse.mybir as mybir
from concourse import bacc
from concourse.masks import make_identity

B, N, D = 8, 1024, 512
H, DV, DQ = 8, 64, 64
E = 2 * H * DV + 2 * H * DQ  # 2048
EPS = 1e-5
P = 128
NT = N // P  # 8 row tiles
F32 = mybir.dt.float32
F16 = mybir.dt.float16
NEG = -10000.0

_cache = {}


def _bucket(d):
    d = np.maximum(np.abs(d), 1).astype(np.float32)
    return np.clip((np.log(d) / 0.301).astype(np.int32), 0, 128)


def _plan_chunks(ts, tsq):
    """Uniform-across-batch k-ranges for the threshold passes."""
    far = []  # (r, n0, n1, kmin, kmax)
    for r in range(NT):
        n0 = P * (r + 2)
        while n0 < N:
            n1 = min(((n0 // 512) + 1) * 512, N)
            dmin = int((tsq[:, n0] - ts[:, P * r + P - 1]).min())
            dmax = int((tsq[:, n1 - 1] - ts[:, P * r]).max())
            far.append((r, n0, n1, int(_bucket(dmin)), int(_bucket(dmax))))
            n0 = n1
    # diag band: n in [128r, 128r+128), cells n >= m only
    dmin_g = int((tsq - ts).min())
    dmax_g = 0
    for r in range(NT):
        dmax_g = max(dmax_g, int((tsq[:, P * r + P - 1] - ts[:, P * r]).max()))
    kmin_g, kmax_g = int(_bucket(max(dmin_g, 0))), int(_bucket(dmax_g))
    # band1: n in [128(r+1), 128(r+2)) for r=0..6
    d1min = min(int((tsq[:, P * (r + 1)] - ts[:, P * r + P - 1]).min()) for r in range(NT - 1))
    d1max = max(int((tsq[:, P * (r + 2) - 1] - ts[:, P * r]).max()) for r in range(NT - 1))
    k1min, k1max = int(_bucket(max(d1min, 0))), int(_bucket(d1max))
    return far, kmin_g, kmax_g, k1min, k1max


def _build(ts_w_np, far, kmin_g, kmax_g, k1min, k1max, ln_trivial=False):
    nc = bacc.Bacc()
    d = {}
    for name, shape, dt in [
        ("xT", [D, N], F16), ("xr", [N, D], F16), ("tsq_row", [1, N], F32),
        ("tsk_col", [P, NT], F32), ("uvqk_g", [D, E], F16),
        ("bU_col", [P, E // P], F32), ("bUv_rep", [P, DV * H], F32),
        ("W_o", [D, D], F16), ("ga_col", [P, 4], F32), ("bb_col", [P, 4], F32),
        ("vscale_col", [P, NT], F32), ("padout_col", [P, NT], F32),
        ("posacc", [P, 4608], F16),
    ]:
        d[name] = nc.dram_tensor(name, shape, dt, kind="ExternalInput")
    out_t = nc.dram_tensor("out", [N, D], F16, kind="ExternalOutput")

    widths = [N - P * r for r in range(NT)]
    offs = np.concatenate([[0], np.cumsum(widths)]).astype(int)
    tsw = ts_w_np.astype(np.float64)
    cks = [float(tsw[k] - tsw[k - 1]) for k in range(1, 129)]
    TH = 2.0 * 0.301  # y = ln(d^2) threshold scale
    TT = mybir.AluOpType

    from contextlib import ExitStack
    with tile.TileContext(nc) as tc, ExitStack() as ctx:
        io = ctx.enter_context(tc.tile_pool(name="io", bufs=1))
        pools = ctx.enter_context(tc.tile_pool(name="work", bufs=4))
        kpool = ctx.enter_context(tc.tile_pool(name="kpool", bufs=9))
        qpool = ctx.enter_context(tc.tile_pool(name="qpool", bufs=4))
        psum = ctx.enter_context(tc.tile_pool(name="psum", bufs=1, space="PSUM"))
        psqk = ctx.enter_context(tc.tile_pool(name="psqk", bufs=3, space="PSUM"))
        psmall = ctx.enter_context(tc.tile_pool(name="psmall", bufs=1, space="PSUM"))

        # ---- persistent SBUF tensors (timestamp path first: it heads the
        # critical DVE threshold chain) ----
        tsq_row = io.tile([1, N], F32, tag="tsqrow")
        nc.sync.dma_start(tsq_row[:], d["tsq_row"][:])
        small = {}
        for nm, sh in [("tsk_col", [P, NT]), ("bU_col", [P, E // P]),
                       ("bUv_rep", [P, DV * H]),
                       ("ga_col", [P, 4]), ("bb_col", [P, 4]),
                       ("vscale_col", [P, NT]), ("padout_col", [P, NT])]:
            small[nm] = io.tile(sh, F32, tag=nm, name=nm)
            nc.sync.dma_start(small[nm][:], d[nm][:])
        tsq_rep = io.tile([P, N], F32, tag="tsqr")
        nc.gpsimd.partition_broadcast(tsq_rep[:], tsq_row[0:1, :])
        xT16 = [io.tile([P, N], F16, tag=f"xS{s}", name=f"xT{s}") for s in range(4)]
        for s in range(4):
            nc.sync.dma_start(xT16[s][:], d["xT"][P * s:P * s + P, :])
        acc = [io.tile([P, widths[r]], F16, tag=f"acc{r}", name=f"acc{r}") for r in range(NT)]
        for r in range(NT):
            nc.sync.dma_start(acc[r][:], d["posacc"][:, offs[r]:offs[r + 1]])

        ident = io.tile([P, P], F16, tag="ident")
        make_identity(nc, ident[:])
        ones_col16 = io.tile([P, 1], F16, tag="ones_col16")
        nc.vector.memset(ones_col16[:], 1.0)

        # ---- layernorm stats of x (over D, partition dim; f16 ones-matmul) ----
        stp = psmall.tile([33, 512], F32, tag="st1", name="stx1")
        stq = psmall.tile([33, 512], F32, tag="st2", name="stx2")
        s1p = [stp[0:1, :], stp[32:33, :]]
        s2p = [stq[0:1, :], stq[32:33, :]]
        # y = ln(dt^2) chain first: it heads the critical DVE threshold chain
        yh = [io.tile([P, widths[r]], F16, tag=f"yh{r}", name=f"yh{r}") for r in range(NT)]
        for r in range(NT):
            w = widths[r]
            db = pools.tile([P, N], F32, tag="w32", name="db")
            d2 = pools.tile([P, N], F32, tag="w32", name="d2")
            nc.vector.tensor_scalar(db[:, :w], tsq_rep[:, P * r:N],
                                    small["tsk_col"][:, r:r + 1], None,
                                    TT.subtract)
            nc.gpsimd.tensor_tensor(d2[:, :w], db[:, :w], db[:, :w], TT.mult)
            nc.scalar.activation(yh[r][:], d2[:, :w],
                                 mybir.ActivationFunctionType.Ln)
        for s in range(4):
            sq = pools.tile([P, N], F16, tag="w16s", name="sq")
            nc.vector.tensor_tensor(sq[:], xT16[s][:], xT16[s][:], TT.mult)
            for c in range(2):
                nc.tensor.matmul(s1p[c][:], ones_col16[:],
                                 xT16[s][:, 512 * c:512 * c + 512],
                                 start=(s == 0), stop=(s == 3))
                nc.tensor.matmul(s2p[c][:], ones_col16[:],
                                 sq[:, 512 * c:512 * c + 512],
                                 start=(s == 0), stop=(s == 3))

        def stats_finalize(s1, s2, tag):
            # mu = s1/D ; rs = 1/sqrt(s2/D - mu^2 + EPS); [1, N] chains,
            # processed per column-half so the two halves pipeline
            mu = pools.tile([1, N], F32, tag="w1n", name=f"mu{tag}")
            rs = pools.tile([1, N], F32, tag="w1n", name=f"rs{tag}")
            t = pools.tile([1, N], F32, tag="w1n", name=f"t{tag}")
            m2 = pools.tile([1, N], F32, tag="w1n", name=f"m2{tag}")
            for c in range(2):
                cc = slice(512 * c, 512 * c + 512)
                nc.scalar.mul(mu[:, cc], s1[c][:], 1.0 / D)
                nc.scalar.mul(t[:, cc], s2[c][:], 1.0 / D)
                nc.vector.tensor_tensor(m2[:, cc], mu[:, cc], mu[:, cc], TT.mult)
                nc.vector.scalar_tensor_tensor(t[:, cc], t[:, cc], EPS, m2[:, cc],
                                               TT.add, TT.subtract)
                nc.scalar.activation(t[:, cc], t[:, cc],
                                     mybir.ActivationFunctionType.Sqrt)
                nc.vector.reciprocal(rs[:, cc], t[:, cc])
            return mu, rs

        mu, rs = stats_finalize(s1p, s2p, "x")

        # replicate mu, rs to [P, N] on gpsimd
        mur = io.tile([P, N], F32, tag="mur")
        rsr = io.tile([P, N], F32, tag="rsr")
        nc.gpsimd.partition_broadcast(mur[:], mu[0:1, :])
        nc.gpsimd.partition_broadcast(rsr[:], rs[0:1, :])

        # ---- stacked y views for the diag / band1 threshold passes ----
        ystack = io.tile([P, N], F16, tag="ystack")
        ystack2 = io.tile([P, N - P], F16, tag="ystack2")
        for r in range(NT):
            nc.vector.tensor_copy(out=ystack[:, P * r:P * r + P], in_=yh[r][:, 0:P])
            if r < NT - 1:
                nc.vector.tensor_copy(out=ystack2[:, P * r:P * r + P], in_=yh[r][:, P:2 * P])

        # far chunk passes: compares on DVE, adds accumulated on PE in PSUM,
        # one DVE add per chunk folds the PSUM sum into the f16 master acc.
        # Emitted first: they unblock the far attention chunks early.
        pdacc = ctx.enter_context(tc.tile_pool(name="pdacc", bufs=1, space="PSUM"))
        xnt = [None] * 4
        for ci, (r, n0, n1, kmin, kmax) in enumerate(far):
            a, b2 = n0 - P * r, n1 - P * r
            w = b2 - a
            ks = list(range(kmin + 1, kmax + 1))
            da = pdacc.tile([P, w], F32, tag=f"da{ci % 2}", name=f"far{ci}")
            for j, k in enumerate(ks):
                t = kpool.tile([P, N], F16, tag="kt")
                nc.vector.tensor_scalar(t[:, :w], yh[r][:, a:b2], float(TH * k),
                                        cks[k - 1], TT.is_ge, TT.mult)
                nc.tensor.matmul(da[:], ident[:], t[:, :w],
                                 start=(j == 0), stop=(j == len(ks) - 1))
            nc.vector.tensor_tensor(acc[r][:, a:b2], acc[r][:, a:b2],
                                    da[:], TT.add)
            # xn'T = (xT - mu) * rs -> f16, interleaved so the proj matmuls
            # unblock while the far passes are still running
            if ci % 2 == 1 and ci // 2 < 4:
                s = ci // 2
                t2 = pools.tile([P, N], F32, tag="w32", name="xc")
                nc.gpsimd.tensor_tensor(t2[:], xT16[s][:], mur[:], TT.subtract)
                xn = io.tile([P, N], F16, tag=f"xS{s}", name=f"xn{s}")
                nc.vector.tensor_tensor(xn[:], t2[:], rsr[:], TT.mult)
                xnt[s] = xn

        for s in range(4):
            if xnt[s] is None:
                t2 = pools.tile([P, N], F32, tag="w32", name="xc")
                nc.gpsimd.tensor_tensor(t2[:], xT16[s][:], mur[:], TT.subtract)
                xn = io.tile([P, N], F16, tag=f"xS{s}", name=f"xn{s}")
                nc.vector.tensor_tensor(xn[:], t2[:], rsr[:], TT.mult)
                xnt[s] = xn

        # ---- proj/v emission units (PE work interleaved into the add stream) ----
        uqk_tiles = [0, 1, 2, 3] + list(range(8, 16))
        projT = {t: io.tile([P, N], F16, tag=f"pT{t}", name=f"pT{t}")
                 for t in uqk_tiles}
        vt = [io.tile([P, D], F16, tag=f"v{r}", name=f"v{r}") for r in range(NT)]
        uvv = []

        def emit_uproj(t, c, uvs):
            pt = psum.tile([P, 512], F32, tag="proj")
            for s in range(4):
                nc.tensor.matmul(pt[:], uvs[s][:],
                                 xnt[s][:, 512 * c:512 * c + 512],
                                 start=(s == 0), stop=(s == 3))
            nc.scalar.activation(projT[t][:, 512 * c:512 * c + 512], pt[:],
                                 mybir.ActivationFunctionType.Silu,
                                 bias=small["bU_col"][:, t:t + 1], scale=1.0)

        def emit_v(r):
            pt = psum.tile([P, 512], F32, tag="proj")
            for s in range(4):
                nc.tensor.matmul(pt[:], xnt[s][:, P * r:P * r + P],
                                 uvv[s][:], start=(s == 0), stop=(s == 3))
            tmpv = pools.tile([P, D], F32, tag="w32", name="tmpv")
            nc.vector.tensor_tensor(tmpv[:], pt[:], small["bUv_rep"][:], TT.add)
            nc.scalar.activation(tmpv[:], tmpv[:], mybir.ActivationFunctionType.Silu)
            nc.gpsimd.tensor_scalar(vt[r][:], tmpv[:], small["vscale_col"][:, r:r + 1],
                                    None, TT.mult)

        # q/k tiles first (attention needs them earliest), then v, then u
        # (only needed by the final LN u-mult)
        units = []
        for t in list(range(8, 16)) + [0, 1, 2, 3]:
            uvs = []
            for s in range(4):
                u1 = pools.tile([P, P], F16, tag="uvs", name="u1")
                nc.sync.dma_start(u1[:], d["uvqk_g"][P * s:P * s + P, P * t:P * t + P])
                uvs.append(u1)
            for c in range(2):
                units.append((emit_uproj, (t, c, uvs)))
        for s in range(4):
            u2 = pools.tile([P, 512], F16, tag="uvv", name="u2")
            nc.sync.dma_start(u2[:], d["uvqk_g"][P * s:P * s + P, 512:1024])
            uvv.append(u2)
        for r in range(NT):
            units.insert(16 + r, (emit_v, (r,)))
        units.reverse()  # pop() from the front

        # ---- diag/band1 threshold passes: compares on DVE, adds on PE into
        # PSUM (identity-matmul accumulate), interleaved with proj units so
        # the PE pipeline stays fed during the DVE chain ----
        dacc16 = io.tile([P, N], F16, tag="dacc")
        dacc216 = io.tile([P, N - P], F16, tag="dacc2")

        def pe_band(ks, src, wtot, out16):
            halves = [(0, 512), (512, wtot)] if wtot > 512 else [(0, wtot)]
            da = [pdacc.tile([P, hb - ha], F32, tag=f"da{i}", name=f"da{i}")
                  for i, (ha, hb) in enumerate(halves)]
            for j, k in enumerate(ks):
                t = kpool.tile([P, N], F16, tag="kt")
                nc.vector.tensor_scalar(t[:, :wtot], src[:], float(TH * k),
                                        cks[k - 1], TT.is_ge, TT.mult)
                for i, (ha, hb) in enumerate(halves):
                    nc.tensor.matmul(da[i][:], ident[:], t[:, ha:hb],
                                     start=(j == 0), stop=(j == len(ks) - 1))
                if j % 3 == 2 and j >= 4 and units:
                    fn, args = units.pop()
                    fn(*args)
            for i, (ha, hb) in enumerate(halves):
                nc.scalar.copy(out=out16[:, ha:hb], in_=da[i][:])

        pe_band(list(range(kmin_g + 1, kmax_g + 1)), ystack[:], N, dacc16)
        for r in range(NT):
            nc.vector.tensor_tensor(acc[r][:, 0:P], acc[r][:, 0:P],
                                    dacc16[:, P * r:P * r + P], TT.add)
            # causal mask within the diagonal block: bias -> -1e4 where m > n,
            # so silu(qk + bias) underflows to 0 for masked cells.
            nc.gpsimd.affine_select(
                out=acc[r][:, 0:P], in_=acc[r][:, 0:P],
                pattern=[[1, P]], compare_op=TT.is_ge, fill=NEG,
                base=0, channel_multiplier=-1)
        pe_band(list(range(k1min + 1, k1max + 1)), ystack2[:], N - P, dacc216)
        for r in range(NT - 1):
            nc.vector.tensor_tensor(acc[r][:, P:2 * P], acc[r][:, P:2 * P],
                                    dacc216[:, P * r:P * r + P], TT.add)
        # flush all but the (late-needed) u units; those interleave into the
        # first attention heads' chunk streams
        while len(units) > 14:
            fn, args = units.pop()
            fn(*args)

        # ---- attention per head (far chunks first within each head) ----
        chunks = []
        for r in range(NT):
            n0 = P * r
            while n0 < N:
                n1 = min(((n0 // 512) + 1) * 512, N)
                chunks.append((r, n0, n1, n0 < P * (r + 2)))
                n0 = n1
        ordered = [c for c in chunks if not c[3]] + [c for c in chunks if c[3]]

        attnT = [io.tile([P, N], F16, tag=f"xS{t}", name=f"aT{t}") for t in range(4)]
        sta = psmall.tile([33, 512], F32, tag="st1", name="sta1")
        stb = psmall.tile([33, 512], F32, tag="st2", name="sta2")
        sa1 = [sta[0:1, :], sta[32:33, :]]
        sa2 = [stb[0:1, :], stb[32:33, :]]
        for h in range(H):
            qt = projT[8 + h // 2]
            kt = projT[12 + h // 2]
            pq = 64 * (h % 2)
            qs = [qpool.tile([P, widths[r]], F16, tag=f"qs{r}", name=f"qs{r}_{h}")
                  for r in range(NT)]
            for ci, (r, n0, n1, near) in enumerate(ordered):
                if ci % 4 == 3:
                    pt = psum.tile([P, 512], F32, tag="proj", name="qk4")
                else:
                    pt = psqk.tile([P, 512], F32, tag="qk")
                cw = n1 - n0
                nc.tensor.matmul(pt[:, :cw], ident[:],
                                 acc[r][:, n0 - P * r:n1 - P * r],
                                 start=True, stop=False)
                nc.tensor.matmul(pt[:, :cw], kt[pq:pq + 64, P * r:P * r + P],
                                 qt[pq:pq + 64, n0:n1], start=False, stop=True)
                nc.scalar.activation(qs[r][:, n0 - P * r:n1 - P * r], pt[:, :cw],
                                     mybir.ActivationFunctionType.Silu)
                if h < 2 and ci % 2 == 1 and units:
                    fn, args = units.pop()
                    fn(*args)
            for c in range(2):
                pa = pdacc.tile([P, 512], F32, tag=f"da{c}", name="av")
                nsub = min(NT, 4 * (c + 1))
                for r in range(nsub):
                    s0 = max(P * r - 512 * c, 0)
                    nc.tensor.matmul(pa[:64, s0:512], vt[r][:, 64 * h:64 * h + 64],
                                     qs[r][:, 512 * c + s0 - P * r:512 * (c + 1) - P * r],
                                     start=(r == 0), stop=(r == nsub - 1))
                at = attnT[h // 2]
                if c == 0:
                    nc.vector.tensor_copy(out=at[pq:pq + 64, 512 * c:512 * c + 512],
                                          in_=pa[:64, :])
                else:
                    nc.scalar.copy(out=at[pq:pq + 64, 512 * c:512 * c + 512],
                                   in_=pa[:64, :])
            if h == 1:
                while units:
                    fn, args = units.pop()
                    fn(*args)
            if h % 2 == 1:
                # attn-LN stats for row-block s = h//2, prepaid inside the
                # attention phase (squares on DVE, sums on PE)
                s = h // 2
                for c in range(2):
                    nc.tensor.matmul(sa1[c][:], ones_col16[:],
                                     attnT[s][:, 512 * c:512 * c + 512],
                                     start=(s == 0), stop=(s == 3))
                    sqa = pools.tile([P, 512], F16, tag="w16a", name="sqa")
                    nc.vector.tensor_tensor(sqa[:], attnT[s][:, 512 * c:512 * c + 512],
                                            attnT[s][:, 512 * c:512 * c + 512], TT.mult)
                    nc.tensor.matmul(sa2[c][:], ones_col16[:], sqa[:],
                                     start=(s == 0), stop=(s == 3))

        while units:
            fn, args = units.pop()
            fn(*args)

        # ---- layernorm of attn (over E=512, partition dim) ----
        mua, rsa = stats_finalize(sa1, sa2, "a")
        mua16 = pools.tile([1, N], F16, tag="w1n16", name="mua16")
        rsa16 = pools.tile([1, N], F16, tag="w1n16", name="rsa16")
        muar = io.tile([P, N], F16, tag="muar")
        rsar = io.tile([P, N], F16, tag="rsar")
        for c in range(2):
            cc = slice(512 * c, 512 * c + 512)
            nc.scalar.copy(out=mua16[:, cc], in_=mua[:, cc])
            nc.scalar.copy(out=rsa16[:, cc], in_=rsa[:, cc])
            nc.gpsimd.partition_broadcast(muar[:, cc], mua16[0:1, cc])
            nc.gpsimd.partition_broadcast(rsar[:, cc], rsa16[0:1, cc])
        # prod = u * (LN_a(attn)*gamma+beta), in attnT layout (f16), applied
        # per column-half so the output projection can start on the first
        # half while the second is still being normalized
        wo = [io.tile([P, D], F16, tag=f"wo{s}", name=f"wo{s}") for s in range(4)]
        for s in range(4):
            nc.sync.dma_start(wo[s][:], d["W_o"][P * s:P * s + P, :])

        def emit_outproj(t):
            po = psqk.tile([P, 512], F32, tag="qk", name="outp")
            for s in range(4):
                nc.tensor.matmul(po[:], attnT[s][:, P * t:P * t + P], wo[s][:],
                                 start=(s == 0), stop=(s == 3))
            xtile = pools.tile([P, D], F16, tag="w16x", name="xtile")
            nc.sync.dma_start(xtile[:], d["xr"][P * t:P * t + P, :])
            ot = pools.tile([P, D], F16, tag="w16x", name="ot")
            nc.vector.scalar_tensor_tensor(ot[:], po[:],
                                           small["padout_col"][:, t:t + 1],
                                           xtile[:], TT.mult, TT.add)
            nc.sync.dma_start(out_t[P * t:P * t + P, :], ot[:])

        for c in range(2):
            cc = slice(512 * c, 512 * c + 512)
            for s in range(4):
                nc.vector.tensor_tensor(attnT[s][:, cc], attnT[s][:, cc],
                                        muar[:, cc], TT.subtract)
                nc.vector.tensor_tensor(attnT[s][:, cc], attnT[s][:, cc],
                                        rsar[:, cc], TT.mult)
                if not ln_trivial:
                    nc.vector.tensor_scalar(attnT[s][:, cc], attnT[s][:, cc],
                                            small["ga_col"][:, s:s + 1],
                                            small["bb_col"][:, s:s + 1],
                                            TT.mult, TT.add)
                nc.vector.tensor_tensor(attnT[s][:, cc], attnT[s][:, cc],
                                        projT[s][:, cc], TT.mult)
            for t in range(4 * c, 4 * c + 4):
                emit_outproj(t)

    nc.compile()
    return nc


def _prep_inputs(inputs):
    x = np.asarray(inputs["x"], dtype=np.float32)
    ts = np.asarray(inputs["timestamps"]).astype(np.int64)
    pad = np.asarray(inputs["pad_mask"]).astype(np.float32)
    uvqk = np.asarray(inputs["uvqk"], dtype=np.float32)
    W_o = np.asarray(inputs["W_o"], dtype=np.float32)
    b_o = np.asarray(inputs["b_o"], dtype=np.float32)
    gx = np.asarray(inputs["gamma_x"], dtype=np.float32)
    bx = np.asarray(inputs["beta_x"], dtype=np.float32)
    ga = np.asarray(inputs["gamma_a"], dtype=np.float32)
    ba = np.asarray(inputs["beta_a"], dtype=np.float32)
    ts_w = np.asarray(inputs["ts_w"], dtype=np.float32)
    pos_w = np.asarray(inputs["pos_w"], dtype=np.float32)

    tsq = np.concatenate([ts[:, 1:], ts[:, -1:]], axis=1)  # [B, N]
    far, kmin_g, kmax_g, k1min, k1max = _plan_chunks(ts, tsq)

    uvqk_g = (uvqk * gx[:, None]).astype(np.float16)
    bU = bx @ uvqk  # [E]
    bU_col = bU.reshape(E // P, P).T.copy()  # [P, E//P]
    bUv_rep = np.broadcast_to(bU[512:1024], (P, 512)).copy()
    ga_col = ga.reshape(4, P).T.copy()
    ba_col = ba.reshape(4, P).T.copy()

    # pos-bias tiles in [m, n] layout + per-chunk base constants
    widths = [N - P * r for r in range(NT)]
    offs = np.concatenate([[0], np.cumsum(widths)]).astype(int)
    posacc = np.zeros((P, int(offs[-1])), np.float32)
    nidx = np.arange(N)
    for r in range(NT):
        m = P * r + np.arange(P)[:, None]
        nn = nidx[None, P * r:]
        posacc[:, offs[r]:offs[r + 1]] = pos_w[nn - m + (N - 1)]
        posacc[:, offs[r]:offs[r] + P] += ts_w[kmin_g]
        if r < NT - 1:
            posacc[:, offs[r] + P:offs[r] + 2 * P] += ts_w[k1min]
    for (r, n0, n1, kmin, kmax) in far:
        posacc[:, offs[r] + n0 - P * r: offs[r] + n1 - P * r] += ts_w[kmin]
    posacc16 = posacc.astype(np.float16)

    per_core = []
    for b in range(B):
        per_core.append({
            "xT": np.ascontiguousarray(x[b].T).astype(np.float16),
            "xr": np.ascontiguousarray(
                (x[b] + b_o[None, :]) * (1.0 - pad[b])[:, None]).astype(np.float16),
            "tsq_row": tsq[b].astype(np.float32).reshape(1, N),
            "tsk_col": np.ascontiguousarray(ts[b].astype(np.float32).reshape(NT, P).T),
            "uvqk_g": uvqk_g, "bU_col": bU_col, "bUv_rep": bUv_rep,
            "W_o": W_o.astype(np.float16),
            "ga_col": ga_col, "bb_col": ba_col,
            "vscale_col": np.ascontiguousarray(
                ((1.0 - pad[b]) / N).astype(np.float32).reshape(NT, P).T),
            "padout_col": np.ascontiguousarray(
                (1.0 - pad[b]).astype(np.float32).reshape(NT, P).T),
            "posacc": posacc16,
        })
    return per_core, (far, kmin_g, kmax_g, k1min, k1max, ts_w)


def kernel(**inputs):
    from concourse.bass_utils import run_bass_kernel_spmd

    per_core, (far, kmin_g, kmax_g, k1min, k1max, ts_w) = _prep_inputs(inputs)
    ln_trivial = bool(
        np.all(np.asarray(inputs["gamma_a"], np.float32) == 1.0)
        and np.all(np.asarray(inputs["beta_a"], np.float32) == 0.0))
    key = (tuple(far), kmin_g, kmax_g, k1min, k1max, ts_w.tobytes(), ln_trivial)
    if key not in _cache:
        _cache.clear()
        _cache[key] = _build(ts_w, far, kmin_g, kmax_g, k1min, k1max, ln_trivial)
    nc = _cache[key]
    res = run_bass_kernel_spmd(nc, per_core, list(range(B)))
    out = np.stack([res.results[b]["out"] for b in range(B)], axis=0)
    return out.astype(np.float32)
